# revision 22
# baseline (speedup 1.0000x reference)
"""EpipolarCrossViewAttention TRN2 kernel v2 (8 NeuronCores, data-parallel).

Sharding: core c -> batch b=c//2, query-row half h=c%2 (1152 query rows).
Host does layout + weight folding + ray normalization (O(N), free);
device does all O(N^2) / O(N*C*D) work.

v2 vs baseline:
- bf16 datapath for q/k/v/P/out projections (validated 5.9e-3 rel).
- fp32-exact top-32 selection (hi/lo f32r bias numerator, fp32 gb,
  128-wide chunk max8 + 4-round merge).
- three overlapped phases: T (bias+topk, DVE/Pool-heavy), P
  (projections, PE-heavy), A (attention, PE/Act), interleaved emission
  so engines pipeline across phases; double-buffered pools.
- masked bias mgb = gb + BIG*min(gb-t,0) precomputed in T (bf16),
  applied in A as Act prefill with per-row -max(gb) shift bias.
- row softmax normalization folded into the O_sb copy (Act scale=R).
"""
import os
import numpy as np
import ml_dtypes
import concourse.bass as bass
import concourse.mybir as mybir
import concourse.tile as tile
from concourse import bacc
from concourse.bass_utils import run_bass_kernel_spmd
from concourse.masks import make_identity

F32 = mybir.dt.float32
F32R = mybir.dt.float32r
BF16 = mybir.dt.bfloat16
A = mybir.AluOpType
AF = mybir.ActivationFunctionType

B, C, H, W = 4, 1024, 48, 48
N = H * W            # 2304 keys
TQ = N // 2          # 1152 query rows per core
D = 256
NC_ = C // 128       # 8 c-tiles
NI = TQ // 128       # 9 query row-blocks
NJ = N // 128        # 18 key 128-chunks
EPS = 1e-6
LN_EPS = 1e-5
SCALE = D ** -0.5
BIG = 1.5e9
TOPCW = 128          # topk chunk width -> 18 chunks, top-8 each

_CACHE = {}
BUILD_ID = 206

KCH = [(0, 512), (512, 512), (1024, 512), (1536, 512), (2048, 256)]  # N chunks
CCH = [(0, 512), (512, 512)]                                         # C chunks


def build_nc():
    nc = bacc.Bacc("TRN2", target_bir_lowering=False, debug=False)

    xq_d = nc.dram_tensor("xq", [C, TQ], BF16, kind="ExternalInput")
    xkv_d = nc.dram_tensor("xkv", [C, N], BF16, kind="ExternalInput")
    q24_d = nc.dram_tensor("q24", [24, TQ], F32R, kind="ExternalInput")  # host hi/lo split
    k24_d = nc.dram_tensor("k24", [24, N], F32R, kind="ExternalInput")
    nkk_d = nc.dram_tensor("nkk", [1, N], F32, kind="ExternalInput")    # -(||mk||+eps)/10
    nqq_d = nc.dram_tensor("nqq", [128, NI], F32, kind="ExternalInput")  # -||mq||/10 per row-block
    wqg_d = nc.dram_tensor("wqg", [C, D], BF16, kind="ExternalInput")   # (Wq*g_q).T * scale
    wkg_d = nc.dram_tensor("wkg", [C, D], BF16, kind="ExternalInput")   # (Wk*g_k).T
    wv_d = nc.dram_tensor("wvt", [C, D], BF16, kind="ExternalInput")    # Wv.T
    wo_d = nc.dram_tensor("wot", [D, C], BF16, kind="ExternalInput")    # Wo.T
    su_d = nc.dram_tensor("su", [128, 8], F32, kind="ExternalInput")    # s_q(2) s_k(2) u_q(2) u_k(2)
    bo_d = nc.dram_tensor("bo128", [128, C], BF16, kind="ExternalInput")  # bo + Wo@bv replicated
    y_d = nc.dram_tensor("y", [TQ, C], F32, kind="ExternalOutput")
    nonce_d = nc.dram_tensor(f"nonce{BUILD_ID}", [1, 1], F32, kind="ExternalInput")
    dnonce_d = nc.dram_tensor(f"dnonce{BUILD_ID}", [1, 1], F32, kind="ExternalOutput")
    DBG = bool(os.environ.get("KDBG"))
    if DBG:
        dbg_gb = nc.dram_tensor("dbg_gb", [128, N], F32, kind="ExternalOutput")
        dbg_t = nc.dram_tensor("dbg_t", [128, 8], F32, kind="ExternalOutput")
        dbg_P = nc.dram_tensor("dbg_P", [128, N], BF16, kind="ExternalOutput")
        dbg_S = nc.dram_tensor("dbg_S", [128, 1], F32, kind="ExternalOutput")
        dbg_mgb = nc.dram_tensor("dbg_mgb", [128, N], BF16, kind="ExternalOutput")
        dbg_s1m = nc.dram_tensor("dbg_s1m", [128, N], BF16, kind="ExternalOutput")

    with tile.TileContext(nc) as tc:
      with tc.tile_pool(name="pers", bufs=1) as pers:
        nt = pers.tile([1, 1], F32, tag="nonce_t")
        nc.sync.dma_start(nt[:], nonce_d[:])
        nc.sync.dma_start(dnonce_d[:], nt[:])

        # geometry + bias inputs first in the DMA queue: phase T needs them
        nqq = pers.tile([128, NI], F32, tag="nqq")
        nc.sync.dma_start(nqq[:], nqq_d[:])
        su = pers.tile([128, 8], F32, tag="su")
        nc.sync.dma_start(su[:], su_d[:])
        nkneg_b = pers.tile([128, N], F32, tag="nkneg_b")

        ident_f = pers.tile([128, 128], F32, tag="ident_f")
        make_identity(nc, ident_f[:])
        ident_r = pers.tile([128, 128], F32R, tag="ident_r")
        nc.vector.tensor_copy(ident_r[:], ident_f[:])
        ident_bf = pers.tile([128, 128], BF16, tag="ident_bf")
        nc.vector.tensor_copy(ident_bf[:], ident_f[:])
        invC = pers.tile([128, 1], BF16, tag="invC")
        nc.vector.memset(invC[:], 1.0 / C)
        lneps = pers.tile([1, 1], F32, tag="lneps")
        nc.vector.memset(lneps[:], LN_EPS)

        q_T = [pers.tile([128, TQ], F32R, tag=f"qT{d}", name=f"qT{d}") for d in range(2)]
        k_T = [pers.tile([128, N], F32R, tag=f"kT{d}", name=f"kT{d}") for d in range(2)]
        V = [pers.tile([128, D + 1], BF16, tag=f"V{t}", name=f"V{t}") for t in range(NJ)]
        for t in range(NJ):
            nc.vector.memset(V[t][:, D:D + 1], 1.0)
        mgb = [pers.tile([128, N], BF16, tag=f"mgb{g}", name=f"mgb{g}") for g in range(NI)]
        gmneg = pers.tile([128, NI], F32, tag="gmneg")
        q24 = pers.tile([24, TQ], F32R, tag="q24")
        k24 = pers.tile([24, N], F32R, tag="k24")

        # ---- geometry fully host-side: direct q24/k24 loads ----
        nc.sync.dma_start(q24[:], q24_d[:])
        nc.sync.dma_start(k24[:], k24_d[:])
        with tc.tile_pool(name="geo", bufs=1) as geo:
            nkrow = geo.tile([1, N], F32, tag="nkrow")
            nc.sync.dma_start(nkrow[:], nkk_d[:])
            nc.gpsimd.partition_broadcast(nkneg_b[:], nkrow[0:1, :], channels=128)

        # weights after geometry in the DMA queue
        wqg = [pers.tile([128, D], BF16, tag=f"wqg{c}", name=f"wqg{c}") for c in range(NC_)]
        wkg = [pers.tile([128, D], BF16, tag=f"wkg{c}", name=f"wkg{c}") for c in range(NC_)]
        wv = [pers.tile([128, D], BF16, tag=f"wv{c}", name=f"wv{c}") for c in range(NC_)]
        wo = [pers.tile([128, C], BF16, tag=f"wo{d}", name=f"wo{d}") for d in range(2)]
        for c in range(NC_):
            nc.sync.dma_start(wqg[c][:], wqg_d[c * 128:(c + 1) * 128, :])
            nc.sync.dma_start(wkg[c][:], wkg_d[c * 128:(c + 1) * 128, :])
            nc.sync.dma_start(wv[c][:], wv_d[c * 128:(c + 1) * 128, :])
        for d in range(2):
            nc.sync.dma_start(wo[d][:], wo_d[d * 128:(d + 1) * 128, :])
        bo_bc = pers.tile([128, C], BF16, tag="bo_bc")
        nc.sync.dma_start(bo_bc[:], bo_d[:])

        # ---- phases T (bias+topk) and P (projections), interleaved ----
        NCAND = (N // TOPCW) * 8   # 144

        with tc.tile_pool(name="psT", bufs=3, space="PSUM") as psT, \
             tc.tile_pool(name="tT", bufs=2) as tT, \
             tc.tile_pool(name="tT1", bufs=1) as tT1, \
             tc.tile_pool(name="tTs", bufs=2) as tTs, \
             tc.tile_pool(name="psS", bufs=1, space="PSUM") as psS, \
             tc.tile_pool(name="psA", bufs=2, space="PSUM") as psA, \
             tc.tile_pool(name="psV", bufs=1, space="PSUM") as psV, \
             tc.tile_pool(name="tP", bufs=2) as tP, \
             tc.tile_pool(name="tPs", bufs=2) as tPs:

            def phase_T_front(g):
                a10 = tT.tile([128, N], F32, tag="a10")
                for j0, wd in KCH:
                    pn = psT.tile([128, 512], F32, tag="pn")
                    nc.tensor.matmul(pn[:, :wd], q24[:, g * 128:(g + 1) * 128],
                                     k24[:, j0:j0 + wd], start=True, stop=True)
                    nc.scalar.activation(a10[:, j0:j0 + wd], pn[:, :wd], AF.Abs)
                dneg = tT.tile([128, N], F32, tag="dneg")
                nc.scalar.activation(dneg[:], nkneg_b[:], AF.Identity,
                                     bias=nqq[:, g:g + 1])
                nc.vector.reciprocal(dneg[:], dneg[:])              # rd in place
                nc.gpsimd.tensor_mul(a10[:], a10[:], dneg[:])       # gb in place
                return a10

            def phase_T_topk(g, gb):
                cand = tT1.tile([128, NCAND], F32, tag="cand")
                for i in range(N // TOPCW):
                    nc.vector.max(out=cand[:, i * 8:(i + 1) * 8],
                                  in_=gb[:, i * TOPCW:(i + 1) * TOPCW])
                scr = tT1.tile([128, NCAND], F32, tag="scr")
                m8s = [tTs.tile([128, 8], F32, tag=f"m8{r}", name=f"m8{r}")
                       for r in range(4)]
                cur = cand
                for r in range(4):
                    nc.vector.max(out=m8s[r][:], in_=cur[:])
                    if r < 3:
                        nxt = scr if cur is cand else cand
                        nc.vector.match_replace(out=nxt[:], in_to_replace=m8s[r][:],
                                                in_values=cur[:], imm_value=-3.0e38)
                        cur = nxt
                nc.vector.tensor_scalar(gmneg[:, g:g + 1], m8s[0][:, 0:1],
                                        -1.0, None, op0=A.mult)
                s1m = tT.tile([128, N], BF16, tag="s1m")
                nc.gpsimd.tensor_scalar(s1m[:], gb[:], m8s[3][:, 7:8], 0.0,
                                        op0=A.subtract, op1=A.min)
                if DBG and g == 0:
                    nc.sync.dma_start(dbg_gb[:], gb[:])
                    nc.sync.dma_start(dbg_t[:], m8s[3][:])
                return s1m

            def emit_mgb(g, gb, s1m):
                nc.vector.scalar_tensor_tensor(mgb[g][:], s1m[:], BIG, gb[:],
                                               op0=A.mult, op1=A.add)

            def phase_P(x_d, j0, wd, out_T, s0, u0, with_v, tok0):
                xt = [tP.tile([128, 512], BF16, tag=f"xt{c}", name=f"xt{c}")
                      for c in range(NC_)]
                for c in range(NC_):
                    nc.sync.dma_start(xt[c][:, :wd],
                                      x_d[c * 128:(c + 1) * 128, j0:j0 + wd])
                p_mu = psS.tile([1, 512], F32, tag="p_mu")
                p_m2 = psS.tile([1, 512], F32, tag="p_m2")
                for c in range(NC_):
                    nc.tensor.matmul(p_mu[:, :wd], invC[:], xt[c][:, :wd],
                                     start=(c == 0), stop=(c == NC_ - 1),
                                     skip_group_check=True)
                    xsq = tPs.tile([128, 512], BF16, tag="xsq")
                    nc.vector.tensor_mul(xsq[:, :wd], xt[c][:, :wd], xt[c][:, :wd])
                    nc.tensor.matmul(p_m2[:, :wd], invC[:], xsq[:, :wd],
                                     start=(c == 0), stop=(c == NC_ - 1),
                                     skip_group_check=True)
                st = tPs.tile([1, 512], F32, tag="st")
                nc.scalar.activation(st[:, :wd], p_mu[:, :wd], AF.Square)   # mu^2
                nc.vector.tensor_sub(st[:, :wd], p_m2[:, :wd], st[:, :wd])  # var
                nc.scalar.activation(st[:, :wd], st[:, :wd], AF.Sqrt,
                                     bias=lneps[0:1, 0:1])                  # sd
                rrow = tPs.tile([1, 512], BF16, tag="rrow")
                mrow = tPs.tile([1, 512], BF16, tag="mrow")
                with nc.allow_low_precision(reason="LN scale rows feed bf16 matmul path"):
                    nc.vector.reciprocal(rrow[:, :wd], st[:, :wd])          # rr (bf16)
                    nc.vector.tensor_mul(mrow[:, :wd], rrow[:, :wd], p_mu[:, :wd])
                rr_b = tPs.tile([128, 512], BF16, tag="rr_b")
                nc.gpsimd.partition_broadcast(rr_b[:, :wd], rrow[0:1, :wd], channels=128)
                m_b = tPs.tile([128, 512], BF16, tag="m_b")
                nc.gpsimd.partition_broadcast(m_b[:, :wd], mrow[0:1, :wd], channels=128)
                for dh in range(2):
                    pA = psA.tile([128, 512], F32, tag="pA")
                    for c in range(NC_):
                        nc.tensor.matmul(pA[:, :wd], wqg[c][:, dh * 128:(dh + 1) * 128]
                                         if out_T is q_T else
                                         wkg[c][:, dh * 128:(dh + 1) * 128],
                                         xt[c][:, :wd], start=(c == 0),
                                         stop=(c == NC_ - 1), skip_group_check=True)
                    pAb = tPs.tile([128, 512], BF16, tag="pAb")
                    nc.scalar.activation(pAb[:, :wd], pA[:, :wd], AF.Identity)
                    k1 = tPs.tile([128, 512], BF16, tag="k1")
                    nc.vector.tensor_mul(k1[:, :wd], pAb[:, :wd], rr_b[:, :wd])
                    k2 = tPs.tile([128, 512], BF16, tag="k2")
                    nc.vector.scalar_tensor_tensor(k2[:, :wd], m_b[:, :wd],
                                                   su[:, s0 + dh:s0 + dh + 1],
                                                   k1[:, :wd], op0=A.mult, op1=A.add)
                    nc.scalar.activation(out_T[dh][:, tok0 + j0:tok0 + j0 + wd],
                                         k2[:, :wd], AF.Identity,
                                         bias=su[:, u0 + dh:u0 + dh + 1])
                if with_v:
                    for s in range(wd // 128):
                        t_idx = (j0 + s * 128) // 128
                        pV = psV.tile([128, D], F32, tag="pV")
                        for c in range(NC_):
                            nc.tensor.matmul(pV[:], xt[c][:, s * 128:(s + 1) * 128],
                                             wv[c][:], start=(c == 0),
                                             stop=(c == NC_ - 1),
                                             skip_group_check=True)
                        nc.scalar.activation(V[t_idx][:, :D], pV[:], AF.Identity)

            # interleave: T(g) then one P chunk-unit
            punits = [("kv", j0, wd) for j0, wd in KCH] + \
                     [("q", j0, wd) for j0, wd in [(0, 512), (512, 512), (1024, 128)]]
            pend = None   # (g, gb, s1m) pipelined by one g: mgb emitted late
            for g in range(NI):
                gb = phase_T_front(g)
                if g < len(punits):
                    kind, j0, wd = punits[g]
                    if kind == "kv":
                        phase_P(xkv_d, j0, wd, k_T, 2, 6, True, 0)
                    else:
                        phase_P(xq_d, j0, wd, q_T, 0, 4, False, 0)
                s1m = phase_T_topk(g, gb)
                if pend is not None:
                    emit_mgb(*pend)
                pend = (g, gb, s1m)
            emit_mgb(*pend)

        # ---- phase A: attention ----
        with tc.tile_pool(name="psL", bufs=2, space="PSUM") as psL, \
             tc.tile_pool(name="psTP", bufs=2, space="PSUM") as psTP, \
             tc.tile_pool(name="psT2", bufs=1, space="PSUM") as psT2, \
             tc.tile_pool(name="psO", bufs=1, space="PSUM") as psO, \
             tc.tile_pool(name="psF", bufs=2, space="PSUM") as psF, \
             tc.tile_pool(name="tA", bufs=2) as tA, \
             tc.tile_pool(name="tAs", bufs=2) as tAs:
            # emgb = exp(mgb - gmax) in place, emitted up front so the Act
            # engine absorbs these during the T+P tail (one exp-table swap)
            for g in range(NI):
                nc.scalar.activation(mgb[g][:], mgb[g][:], AF.Exp,
                                     bias=gmneg[:, g:g + 1])
            for g in range(NI):
                P = tA.tile([128, N], BF16, tag="P")
                for ci, (j0, wd) in enumerate(KCH):
                    pL = psL.tile([128, 512], F32, tag="pL")
                    for dh in range(2):
                        nc.tensor.matmul(pL[:, :wd],
                                         q_T[dh][:, g * 128:(g + 1) * 128],
                                         k_T[dh][:, j0:j0 + wd],
                                         start=(dh == 0), stop=(dh == 1),
                                         skip_group_check=True)
                    nc.scalar.activation(P[:, j0:j0 + wd], pL[:, :wd], AF.Exp)
                nc.vector.tensor_mul(P[:], P[:], mgb[g][:])   # masked P, bf16 4x
                if DBG and g == 0:
                    nc.sync.dma_start(dbg_P[:], P[:])

                pO = psO.tile([128, D + 1], F32, tag="pO")
                for grp in range(5):  # groups of 4 transposes (last group 2)
                    njg = 4 if grp < 4 else 2
                    ptp = psTP.tile([128, 512], BF16, tag="ptp")
                    for jj in range(njg):
                        j = grp * 4 + jj
                        nc.tensor.transpose(ptp[:, jj * 128:(jj + 1) * 128],
                                            P[:, j * 128:(j + 1) * 128], ident_bf[:])
                    Pt = tAs.tile([128, 512], BF16, tag="Pt")
                    if grp % 2 == 0:
                        nc.scalar.activation(Pt[:, :njg * 128], ptp[:, :njg * 128],
                                             AF.Identity)
                    else:
                        nc.vector.tensor_scalar(Pt[:, :njg * 128], ptp[:, :njg * 128],
                                                1.0, None, op0=A.mult)
                    for jj in range(njg):
                        j = grp * 4 + jj
                        nc.tensor.matmul(pO[:], Pt[:, jj * 128:(jj + 1) * 128],
                                         V[j][:], start=(j == 0), stop=(j == NJ - 1),
                                         skip_group_check=True)
                R = tAs.tile([128, 1], F32, tag="R")
                nc.vector.reciprocal(R[:], pO[:, D:D + 1])
                O_sb = tAs.tile([128, D], BF16, tag="O_sb")
                nc.scalar.activation(O_sb[:], pO[:, :D], AF.Identity, scale=R[:, 0:1])

                ptp2 = psT2.tile([128, 256], BF16, tag="ptp2")
                for dh in range(2):
                    nc.tensor.transpose(ptp2[:, dh * 128:(dh + 1) * 128],
                                        O_sb[:, dh * 128:(dh + 1) * 128], ident_bf[:])
                OT = tAs.tile([128, D], BF16, tag="OT")
                nc.vector.tensor_scalar(OT[:], ptp2[:, 0:D], 1.0, None, op0=A.mult)
                for j0, wd in CCH:
                    pF = psF.tile([128, 512], F32, tag="pF")
                    for dh in range(2):
                        nc.tensor.matmul(pF[:, :wd], OT[:, dh * 128:(dh + 1) * 128],
                                         wo[dh][:, j0:j0 + wd], start=(dh == 0),
                                         stop=(dh == 1), skip_group_check=True)
                    fo = tA.tile([128, 512], F32, tag="fo")
                    nc.vector.tensor_add(fo[:, :wd], pF[:, :wd], bo_bc[:, j0:j0 + wd])
                    nc.sync.dma_start(y_d[g * 128:(g + 1) * 128, j0:j0 + wd], fo[:, :wd])

    nc.finalize()
    return nc


def _host_inputs(inputs):
    f32 = np.float32
    qm = np.asarray(inputs["query_map"], f32).reshape(B, C, N)
    kv = np.asarray(inputs["key_value_map"], f32).reshape(B, C, N)
    pq = np.asarray(inputs["plucker_query"], f32).reshape(B, 6, N)
    pk = np.asarray(inputs["plucker_key"], f32).reshape(B, 6, N)
    Wq, Wk, Wv, Wo = (np.asarray(inputs[k], f32) for k in ("Wq", "Wk", "Wv", "Wo"))
    gq, bq_ln = np.asarray(inputs["ln_q_g"], f32), np.asarray(inputs["ln_q_b"], f32)
    gk, bk_ln = np.asarray(inputs["ln_k_g"], f32), np.asarray(inputs["ln_k_b"], f32)
    bq, bk, bv, bo = (np.asarray(inputs[k], f32) for k in ("bq", "bk", "bv", "bo"))

    bf = ml_dtypes.bfloat16
    wqg = ((Wq * gq[None, :]).T * SCALE).astype(bf)          # [C, D]
    wkg = (Wk * gk[None, :]).T.astype(bf)
    wvt = Wv.T.astype(bf)
    wot = Wo.T.astype(bf)
    u_q = ((Wq @ bq_ln + bq) * SCALE).astype(f32)            # [D]
    u_k = (Wk @ bk_ln + bk).astype(f32)
    s_q = -wqg.astype(f32).sum(axis=0)                       # [D]
    s_k = -wkg.astype(f32).sum(axis=0)
    su = np.zeros((128, 8), f32)
    for dh in range(2):
        su[:, 0 + dh] = s_q[dh * 128:(dh + 1) * 128]
        su[:, 2 + dh] = s_k[dh * 128:(dh + 1) * 128]
        su[:, 4 + dh] = u_q[dh * 128:(dh + 1) * 128]
        su[:, 6 + dh] = u_k[dh * 128:(dh + 1) * 128]
    bo_row = (bo + Wo @ bv).astype(f32)
    bo128 = np.broadcast_to(bo_row[None, :].astype(bf), (128, C))

    # geometry: normalized dirs + moments + norms + f32r hi/lo split
    def rn11(x):
        b = np.ascontiguousarray(x, f32).view(np.uint32)
        return ((b + np.uint32(1 << 11)) & np.uint32(0xFFFFF000)).view(f32)

    def geo(p, qside):  # p [6, M]
        d = p[0:3]; m = p[3:6]
        nd = np.linalg.norm(d, axis=0)
        dn = d / np.maximum(nd, EPS)[None, :]
        nm = np.linalg.norm(m, axis=0)
        p6 = np.concatenate([dn, m], axis=0).astype(f32)
        hi = rn11(p6); lo = (p6 - hi).astype(f32)
        t24 = np.zeros((24, p6.shape[1]), f32)
        if qside:
            for base, src in ((0, lo), (6, lo), (12, hi), (18, hi)):
                t24[base:base + 3] = src[0:3]
                t24[base + 3:base + 6] = src[3:6]
        else:
            for base, src in ((0, lo), (6, hi), (12, lo), (18, hi)):
                t24[base:base + 3] = src[3:6]
                t24[base + 3:base + 6] = src[0:3]
        return t24, nm

    in_maps = []
    for core in range(8):
        b, h = core // 2, core % 2
        sl = slice(h * TQ, (h + 1) * TQ)
        q24h, nmq = geo(pq[b][:, sl], True)
        k24h, nmk = geo(pk[b], False)
        nkk = (-(nmk + EPS) / 10.0)[None, :]
        nqq = (-nmq / 10.0).reshape(NI, 128).T       # [128, NI]
        m = {
            "xq": qm[b][:, sl].astype(bf),
            "xkv": kv[b].astype(bf),
            "q24": q24h, "k24": k24h,
            "nkk": nkk.astype(f32), "nqq": nqq.astype(f32),
            "wqg": wqg, "wkg": wkg, "wvt": wvt, "wot": wot,
            "su": su, "bo128": bo128,
            f"nonce{BUILD_ID}": np.zeros((1, 1), f32),
        }
        in_maps.append({k: np.ascontiguousarray(v) for k, v in m.items()})
    return in_maps


def kernel(**inputs):
    if "nc" not in _CACHE:
        _CACHE["nc"] = build_nc()
    nc = _CACHE["nc"]
    in_maps = _host_inputs(inputs)
    res = run_bass_kernel_spmd(nc, in_maps, core_ids=list(range(8)))
    out = np.zeros((B, C, N), np.float32)
    for core in range(8):
        b, h = core // 2, core % 2
        out[b][:, h * TQ:(h + 1) * TQ] = res.results[core]["y"].T
    return out.reshape(B, C, H, W)


# revision 23
# speedup vs baseline: 1.0017x; 1.0017x over previous
"""EpipolarCrossViewAttention TRN2 kernel v2 (8 NeuronCores, data-parallel).

Sharding: core c -> batch b=c//2, query-row half h=c%2 (1152 query rows).
Host does layout + weight folding + ray normalization (O(N), free);
device does all O(N^2) / O(N*C*D) work.

v2 vs baseline:
- bf16 datapath for q/k/v/P/out projections (validated 5.9e-3 rel).
- fp32-exact top-32 selection (hi/lo f32r bias numerator, fp32 gb,
  128-wide chunk max8 + 4-round merge).
- three overlapped phases: T (bias+topk, DVE/Pool-heavy), P
  (projections, PE-heavy), A (attention, PE/Act), interleaved emission
  so engines pipeline across phases; double-buffered pools.
- masked bias mgb = gb + BIG*min(gb-t,0) precomputed in T (bf16),
  applied in A as Act prefill with per-row -max(gb) shift bias.
- row softmax normalization folded into the O_sb copy (Act scale=R).
"""
import os
import numpy as np
import ml_dtypes
import concourse.bass as bass
import concourse.mybir as mybir
import concourse.tile as tile
from concourse import bacc
from concourse.bass_utils import run_bass_kernel_spmd
from concourse.masks import make_identity

F32 = mybir.dt.float32
F32R = mybir.dt.float32r
BF16 = mybir.dt.bfloat16
A = mybir.AluOpType
AF = mybir.ActivationFunctionType

B, C, H, W = 4, 1024, 48, 48
N = H * W            # 2304 keys
TQ = N // 2          # 1152 query rows per core
D = 256
NC_ = C // 128       # 8 c-tiles
NI = TQ // 128       # 9 query row-blocks
NJ = N // 128        # 18 key 128-chunks
EPS = 1e-6
LN_EPS = 1e-5
SCALE = D ** -0.5
BIG = 1.5e9
TOPCW = 128          # topk chunk width -> 18 chunks, top-8 each

_CACHE = {}
BUILD_ID = 206

KCH = [(0, 512), (512, 512), (1024, 512), (1536, 512), (2048, 256)]  # N chunks
CCH = [(0, 512), (512, 512)]                                         # C chunks


def build_nc():
    nc = bacc.Bacc("TRN2", target_bir_lowering=False, debug=False)

    xq_d = nc.dram_tensor("xq", [C, TQ], BF16, kind="ExternalInput")
    xkv_d = nc.dram_tensor("xkv", [C, N], BF16, kind="ExternalInput")
    q24_d = nc.dram_tensor("q24", [24, TQ], F32R, kind="ExternalInput")  # host hi/lo split
    k24_d = nc.dram_tensor("k24", [24, N], F32R, kind="ExternalInput")
    nkk_d = nc.dram_tensor("nkk", [1, N], F32, kind="ExternalInput")    # -(||mk||+eps)/10
    nqq_d = nc.dram_tensor("nqq", [128, NI], F32, kind="ExternalInput")  # -||mq||/10 per row-block
    wqg_d = nc.dram_tensor("wqg", [C, D], BF16, kind="ExternalInput")   # (Wq*g_q).T * scale
    wkg_d = nc.dram_tensor("wkg", [C, D], BF16, kind="ExternalInput")   # (Wk*g_k).T
    wv_d = nc.dram_tensor("wvt", [C, D], BF16, kind="ExternalInput")    # Wv.T
    wo_d = nc.dram_tensor("wot", [D, C], BF16, kind="ExternalInput")    # Wo.T
    su_d = nc.dram_tensor("su", [128, 8], F32, kind="ExternalInput")    # s_q(2) s_k(2) u_q(2) u_k(2)
    bo_d = nc.dram_tensor("bo128", [128, C], BF16, kind="ExternalInput")  # bo + Wo@bv replicated
    y_d = nc.dram_tensor("y", [TQ, C], F32, kind="ExternalOutput")
    nonce_d = nc.dram_tensor(f"nonce{BUILD_ID}", [1, 1], F32, kind="ExternalInput")
    dnonce_d = nc.dram_tensor(f"dnonce{BUILD_ID}", [1, 1], F32, kind="ExternalOutput")
    DBG = bool(os.environ.get("KDBG"))
    if DBG:
        dbg_gb = nc.dram_tensor("dbg_gb", [128, N], F32, kind="ExternalOutput")
        dbg_t = nc.dram_tensor("dbg_t", [128, 8], F32, kind="ExternalOutput")
        dbg_P = nc.dram_tensor("dbg_P", [128, N], BF16, kind="ExternalOutput")
        dbg_S = nc.dram_tensor("dbg_S", [128, 1], F32, kind="ExternalOutput")
        dbg_mgb = nc.dram_tensor("dbg_mgb", [128, N], BF16, kind="ExternalOutput")
        dbg_s1m = nc.dram_tensor("dbg_s1m", [128, N], BF16, kind="ExternalOutput")

    with tile.TileContext(nc) as tc:
      with tc.tile_pool(name="pers", bufs=1) as pers:
        nt = pers.tile([1, 1], F32, tag="nonce_t")
        nc.sync.dma_start(nt[:], nonce_d[:])
        nc.sync.dma_start(dnonce_d[:], nt[:])

        # geometry + bias inputs first in the DMA queue: phase T needs them
        nqq = pers.tile([128, NI], F32, tag="nqq")
        nc.sync.dma_start(nqq[:], nqq_d[:])
        su = pers.tile([128, 8], F32, tag="su")
        nc.sync.dma_start(su[:], su_d[:])
        nkneg_b = pers.tile([128, N], F32, tag="nkneg_b")

        ident_f = pers.tile([128, 128], F32, tag="ident_f")
        make_identity(nc, ident_f[:])
        ident_r = pers.tile([128, 128], F32R, tag="ident_r")
        nc.vector.tensor_copy(ident_r[:], ident_f[:])
        ident_bf = pers.tile([128, 128], BF16, tag="ident_bf")
        nc.vector.tensor_copy(ident_bf[:], ident_f[:])
        invC = pers.tile([128, 1], BF16, tag="invC")
        nc.vector.memset(invC[:], 1.0 / C)
        lneps = pers.tile([1, 1], F32, tag="lneps")
        nc.vector.memset(lneps[:], LN_EPS)

        q_T = [pers.tile([128, TQ], F32R, tag=f"qT{d}", name=f"qT{d}") for d in range(2)]
        k_T = [pers.tile([128, N], F32R, tag=f"kT{d}", name=f"kT{d}") for d in range(2)]
        V = [pers.tile([128, D + 1], BF16, tag=f"V{t}", name=f"V{t}") for t in range(NJ)]
        for t in range(NJ):
            nc.vector.memset(V[t][:, D:D + 1], 1.0)
        mgb = [pers.tile([128, N], BF16, tag=f"mgb{g}", name=f"mgb{g}") for g in range(NI)]
        gmneg = pers.tile([128, NI], F32, tag="gmneg")
        q24 = pers.tile([24, TQ], F32R, tag="q24")
        k24 = pers.tile([24, N], F32R, tag="k24")

        # ---- geometry fully host-side: direct q24/k24 loads ----
        nc.sync.dma_start(q24[:], q24_d[:])
        nc.sync.dma_start(k24[:], k24_d[:])
        with tc.tile_pool(name="geo", bufs=1) as geo:
            nkrow = geo.tile([1, N], F32, tag="nkrow")
            nc.sync.dma_start(nkrow[:], nkk_d[:])
            nc.gpsimd.partition_broadcast(nkneg_b[:], nkrow[0:1, :], channels=128)

        # weights after geometry in the DMA queue
        wqg = [pers.tile([128, D], BF16, tag=f"wqg{c}", name=f"wqg{c}") for c in range(NC_)]
        wkg = [pers.tile([128, D], BF16, tag=f"wkg{c}", name=f"wkg{c}") for c in range(NC_)]
        wv = [pers.tile([128, D], BF16, tag=f"wv{c}", name=f"wv{c}") for c in range(NC_)]
        wo = [pers.tile([128, C], BF16, tag=f"wo{d}", name=f"wo{d}") for d in range(2)]
        for c in range(NC_):
            nc.sync.dma_start(wqg[c][:], wqg_d[c * 128:(c + 1) * 128, :])
            nc.sync.dma_start(wkg[c][:], wkg_d[c * 128:(c + 1) * 128, :])
            nc.sync.dma_start(wv[c][:], wv_d[c * 128:(c + 1) * 128, :])
        for d in range(2):
            nc.sync.dma_start(wo[d][:], wo_d[d * 128:(d + 1) * 128, :])
        bo_bc = pers.tile([128, C], BF16, tag="bo_bc")
        nc.sync.dma_start(bo_bc[:], bo_d[:])

        # ---- phases T (bias+topk) and P (projections), interleaved ----
        NCAND = (N // TOPCW) * 8   # 144

        with tc.tile_pool(name="psT", bufs=3, space="PSUM") as psT, \
             tc.tile_pool(name="tT", bufs=2) as tT, \
             tc.tile_pool(name="tT1", bufs=1) as tT1, \
             tc.tile_pool(name="tTs", bufs=2) as tTs, \
             tc.tile_pool(name="psS", bufs=1, space="PSUM") as psS, \
             tc.tile_pool(name="psA", bufs=2, space="PSUM") as psA, \
             tc.tile_pool(name="psV", bufs=1, space="PSUM") as psV, \
             tc.tile_pool(name="tP", bufs=2) as tP, \
             tc.tile_pool(name="tPs", bufs=2) as tPs:

            def phase_T_front(g):
                a10 = tT.tile([128, N], F32, tag="a10")
                for j0, wd in KCH:
                    pn = psT.tile([128, 512], F32, tag="pn")
                    nc.tensor.matmul(pn[:, :wd], q24[:, g * 128:(g + 1) * 128],
                                     k24[:, j0:j0 + wd], start=True, stop=True)
                    nc.scalar.activation(a10[:, j0:j0 + wd], pn[:, :wd], AF.Abs)
                dneg = tT.tile([128, N], F32, tag="dneg")
                nc.scalar.activation(dneg[:], nkneg_b[:], AF.Identity,
                                     bias=nqq[:, g:g + 1])
                nc.vector.reciprocal(dneg[:], dneg[:])              # rd in place
                nc.gpsimd.tensor_mul(a10[:], a10[:], dneg[:])       # gb in place
                return a10

            def phase_T_topk(g, gb):
                cand = tT1.tile([128, NCAND], F32, tag="cand")
                for i in range(N // TOPCW):
                    nc.vector.max(out=cand[:, i * 8:(i + 1) * 8],
                                  in_=gb[:, i * TOPCW:(i + 1) * TOPCW])
                scr = tT1.tile([128, NCAND], F32, tag="scr")
                m8s = [tTs.tile([128, 8], F32, tag=f"m8{r}", name=f"m8{r}")
                       for r in range(4)]
                cur = cand
                for r in range(4):
                    nc.vector.max(out=m8s[r][:], in_=cur[:])
                    if r < 3:
                        nxt = scr if cur is cand else cand
                        nc.vector.match_replace(out=nxt[:], in_to_replace=m8s[r][:],
                                                in_values=cur[:], imm_value=-3.0e38)
                        cur = nxt
                nc.vector.tensor_scalar(gmneg[:, g:g + 1], m8s[0][:, 0:1],
                                        -1.0, None, op0=A.mult)
                s1m = tT.tile([128, N], BF16, tag="s1m")
                nc.gpsimd.tensor_scalar(s1m[:], gb[:], m8s[3][:, 7:8], 0.0,
                                        op0=A.subtract, op1=A.min)
                if DBG and g == 0:
                    nc.sync.dma_start(dbg_gb[:], gb[:])
                    nc.sync.dma_start(dbg_t[:], m8s[3][:])
                return s1m

            def emit_mgb(g, gb, s1m):
                nc.vector.scalar_tensor_tensor(mgb[g][:], s1m[:], BIG, gb[:],
                                               op0=A.mult, op1=A.add)

            def phase_P(x_d, j0, wd, out_T, s0, u0, with_v, tok0):
                xt = [tP.tile([128, 512], BF16, tag=f"xt{c}", name=f"xt{c}")
                      for c in range(NC_)]
                for c in range(NC_):
                    nc.sync.dma_start(xt[c][:, :wd],
                                      x_d[c * 128:(c + 1) * 128, j0:j0 + wd])
                p_mu = psS.tile([1, 512], F32, tag="p_mu")
                p_m2 = psS.tile([1, 512], F32, tag="p_m2")
                for c in range(NC_):
                    nc.tensor.matmul(p_mu[:, :wd], invC[:], xt[c][:, :wd],
                                     start=(c == 0), stop=(c == NC_ - 1),
                                     skip_group_check=True)
                    xsq = tPs.tile([128, 512], BF16, tag="xsq")
                    nc.vector.tensor_mul(xsq[:, :wd], xt[c][:, :wd], xt[c][:, :wd])
                    nc.tensor.matmul(p_m2[:, :wd], invC[:], xsq[:, :wd],
                                     start=(c == 0), stop=(c == NC_ - 1),
                                     skip_group_check=True)
                st = tPs.tile([1, 512], F32, tag="st")
                nc.scalar.activation(st[:, :wd], p_mu[:, :wd], AF.Square)   # mu^2
                nc.vector.tensor_sub(st[:, :wd], p_m2[:, :wd], st[:, :wd])  # var
                nc.scalar.activation(st[:, :wd], st[:, :wd], AF.Sqrt,
                                     bias=lneps[0:1, 0:1])                  # sd
                rrow = tPs.tile([1, 512], BF16, tag="rrow")
                mrow = tPs.tile([1, 512], BF16, tag="mrow")
                with nc.allow_low_precision(reason="LN scale rows feed bf16 matmul path"):
                    nc.vector.reciprocal(rrow[:, :wd], st[:, :wd])          # rr (bf16)
                    nc.vector.tensor_mul(mrow[:, :wd], rrow[:, :wd], p_mu[:, :wd])
                rr_b = tPs.tile([128, 512], BF16, tag="rr_b")
                nc.gpsimd.partition_broadcast(rr_b[:, :wd], rrow[0:1, :wd], channels=128)
                m_b = tPs.tile([128, 512], BF16, tag="m_b")
                nc.gpsimd.partition_broadcast(m_b[:, :wd], mrow[0:1, :wd], channels=128)
                for dh in range(2):
                    pA = psA.tile([128, 512], F32, tag="pA")
                    for c in range(NC_):
                        nc.tensor.matmul(pA[:, :wd], wqg[c][:, dh * 128:(dh + 1) * 128]
                                         if out_T is q_T else
                                         wkg[c][:, dh * 128:(dh + 1) * 128],
                                         xt[c][:, :wd], start=(c == 0),
                                         stop=(c == NC_ - 1), skip_group_check=True)
                    pAb = tPs.tile([128, 512], BF16, tag="pAb")
                    nc.scalar.activation(pAb[:, :wd], pA[:, :wd], AF.Identity)
                    k1 = tPs.tile([128, 512], BF16, tag="k1")
                    nc.vector.tensor_mul(k1[:, :wd], pAb[:, :wd], rr_b[:, :wd])
                    k2 = tPs.tile([128, 512], BF16, tag="k2")
                    nc.vector.scalar_tensor_tensor(k2[:, :wd], m_b[:, :wd],
                                                   su[:, s0 + dh:s0 + dh + 1],
                                                   k1[:, :wd], op0=A.mult, op1=A.add)
                    nc.scalar.activation(out_T[dh][:, tok0 + j0:tok0 + j0 + wd],
                                         k2[:, :wd], AF.Identity,
                                         bias=su[:, u0 + dh:u0 + dh + 1])
                if with_v:
                    for s in range(wd // 128):
                        t_idx = (j0 + s * 128) // 128
                        pV = psV.tile([128, D], F32, tag="pV")
                        for c in range(NC_):
                            nc.tensor.matmul(pV[:], xt[c][:, s * 128:(s + 1) * 128],
                                             wv[c][:], start=(c == 0),
                                             stop=(c == NC_ - 1),
                                             skip_group_check=True)
                        nc.scalar.activation(V[t_idx][:, :D], pV[:], AF.Identity)

            # interleave: T(g) then one P chunk-unit
            punits = [("kv", j0, wd) for j0, wd in KCH] + \
                     [("q", j0, wd) for j0, wd in [(0, 512), (512, 512), (1024, 128)]]
            pend = None   # (g, gb, s1m) pipelined by one g: mgb emitted late
            for g in range(NI):
                gb = phase_T_front(g)
                if g < len(punits):
                    kind, j0, wd = punits[g]
                    if kind == "kv":
                        phase_P(xkv_d, j0, wd, k_T, 2, 6, True, 0)
                    else:
                        phase_P(xq_d, j0, wd, q_T, 0, 4, False, 0)
                s1m = phase_T_topk(g, gb)
                if pend is not None:
                    emit_mgb(*pend)
                pend = (g, gb, s1m)
            emit_mgb(*pend)

        # ---- phase A: attention ----
        with tc.tile_pool(name="psL", bufs=2, space="PSUM") as psL, \
             tc.tile_pool(name="psTP", bufs=2, space="PSUM") as psTP, \
             tc.tile_pool(name="psT2", bufs=1, space="PSUM") as psT2, \
             tc.tile_pool(name="psO", bufs=1, space="PSUM") as psO, \
             tc.tile_pool(name="psF", bufs=2, space="PSUM") as psF, \
             tc.tile_pool(name="tA", bufs=2) as tA, \
             tc.tile_pool(name="tAs", bufs=2) as tAs:
            for g in range(NI):
                # emgb = exp(mgb - gmax) in place (phase A: exp table resident)
                nc.scalar.activation(mgb[g][:], mgb[g][:], AF.Exp,
                                     bias=gmneg[:, g:g + 1])
                P = tA.tile([128, N], BF16, tag="P")
                for ci, (j0, wd) in enumerate(KCH):
                    pL = psL.tile([128, 512], F32, tag="pL")
                    for dh in range(2):
                        nc.tensor.matmul(pL[:, :wd],
                                         q_T[dh][:, g * 128:(g + 1) * 128],
                                         k_T[dh][:, j0:j0 + wd],
                                         start=(dh == 0), stop=(dh == 1),
                                         skip_group_check=True)
                    nc.scalar.activation(P[:, j0:j0 + wd], pL[:, :wd], AF.Exp)
                nc.vector.tensor_mul(P[:], P[:], mgb[g][:])   # masked P, bf16 4x
                if DBG and g == 0:
                    nc.sync.dma_start(dbg_P[:], P[:])

                pO = psO.tile([128, D + 1], F32, tag="pO")
                for grp in range(5):  # groups of 4 transposes (last group 2)
                    njg = 4 if grp < 4 else 2
                    ptp = psTP.tile([128, 512], BF16, tag="ptp")
                    for jj in range(njg):
                        j = grp * 4 + jj
                        nc.tensor.transpose(ptp[:, jj * 128:(jj + 1) * 128],
                                            P[:, j * 128:(j + 1) * 128], ident_bf[:])
                    Pt = tAs.tile([128, 512], BF16, tag="Pt")
                    if grp % 2 == 0:
                        nc.scalar.activation(Pt[:, :njg * 128], ptp[:, :njg * 128],
                                             AF.Identity)
                    else:
                        nc.vector.tensor_scalar(Pt[:, :njg * 128], ptp[:, :njg * 128],
                                                1.0, None, op0=A.mult)
                    for jj in range(njg):
                        j = grp * 4 + jj
                        nc.tensor.matmul(pO[:], Pt[:, jj * 128:(jj + 1) * 128],
                                         V[j][:], start=(j == 0), stop=(j == NJ - 1),
                                         skip_group_check=True)
                R = tAs.tile([128, 1], F32, tag="R")
                nc.vector.reciprocal(R[:], pO[:, D:D + 1])
                O_sb = tAs.tile([128, D], BF16, tag="O_sb")
                nc.scalar.activation(O_sb[:], pO[:, :D], AF.Identity, scale=R[:, 0:1])

                ptp2 = psT2.tile([128, 256], BF16, tag="ptp2")
                for dh in range(2):
                    nc.tensor.transpose(ptp2[:, dh * 128:(dh + 1) * 128],
                                        O_sb[:, dh * 128:(dh + 1) * 128], ident_bf[:])
                OT = tAs.tile([128, D], BF16, tag="OT")
                nc.vector.tensor_scalar(OT[:], ptp2[:, 0:D], 1.0, None, op0=A.mult)
                for j0, wd in CCH:
                    pF = psF.tile([128, 512], F32, tag="pF")
                    for dh in range(2):
                        nc.tensor.matmul(pF[:, :wd], OT[:, dh * 128:(dh + 1) * 128],
                                         wo[dh][:, j0:j0 + wd], start=(dh == 0),
                                         stop=(dh == 1), skip_group_check=True)
                    fo = tA.tile([128, 512], F32, tag="fo")
                    nc.vector.tensor_add(fo[:, :wd], pF[:, :wd], bo_bc[:, j0:j0 + wd])
                    nc.sync.dma_start(y_d[g * 128:(g + 1) * 128, j0:j0 + wd], fo[:, :wd])

    nc.finalize()
    return nc


def _host_inputs(inputs):
    f32 = np.float32
    qm = np.asarray(inputs["query_map"], f32).reshape(B, C, N)
    kv = np.asarray(inputs["key_value_map"], f32).reshape(B, C, N)
    pq = np.asarray(inputs["plucker_query"], f32).reshape(B, 6, N)
    pk = np.asarray(inputs["plucker_key"], f32).reshape(B, 6, N)
    Wq, Wk, Wv, Wo = (np.asarray(inputs[k], f32) for k in ("Wq", "Wk", "Wv", "Wo"))
    gq, bq_ln = np.asarray(inputs["ln_q_g"], f32), np.asarray(inputs["ln_q_b"], f32)
    gk, bk_ln = np.asarray(inputs["ln_k_g"], f32), np.asarray(inputs["ln_k_b"], f32)
    bq, bk, bv, bo = (np.asarray(inputs[k], f32) for k in ("bq", "bk", "bv", "bo"))

    bf = ml_dtypes.bfloat16
    wqg = ((Wq * gq[None, :]).T * SCALE).astype(bf)          # [C, D]
    wkg = (Wk * gk[None, :]).T.astype(bf)
    wvt = Wv.T.astype(bf)
    wot = Wo.T.astype(bf)
    u_q = ((Wq @ bq_ln + bq) * SCALE).astype(f32)            # [D]
    u_k = (Wk @ bk_ln + bk).astype(f32)
    s_q = -wqg.astype(f32).sum(axis=0)                       # [D]
    s_k = -wkg.astype(f32).sum(axis=0)
    su = np.zeros((128, 8), f32)
    for dh in range(2):
        su[:, 0 + dh] = s_q[dh * 128:(dh + 1) * 128]
        su[:, 2 + dh] = s_k[dh * 128:(dh + 1) * 128]
        su[:, 4 + dh] = u_q[dh * 128:(dh + 1) * 128]
        su[:, 6 + dh] = u_k[dh * 128:(dh + 1) * 128]
    bo_row = (bo + Wo @ bv).astype(f32)
    bo128 = np.broadcast_to(bo_row[None, :].astype(bf), (128, C))

    # geometry: normalized dirs + moments + norms + f32r hi/lo split
    def rn11(x):
        b = np.ascontiguousarray(x, f32).view(np.uint32)
        return ((b + np.uint32(1 << 11)) & np.uint32(0xFFFFF000)).view(f32)

    def geo(p, qside):  # p [6, M]
        d = p[0:3]; m = p[3:6]
        nd = np.linalg.norm(d, axis=0)
        dn = d / np.maximum(nd, EPS)[None, :]
        nm = np.linalg.norm(m, axis=0)
        p6 = np.concatenate([dn, m], axis=0).astype(f32)
        hi = rn11(p6); lo = (p6 - hi).astype(f32)
        t24 = np.zeros((24, p6.shape[1]), f32)
        if qside:
            for base, src in ((0, lo), (6, lo), (12, hi), (18, hi)):
                t24[base:base + 3] = src[0:3]
                t24[base + 3:base + 6] = src[3:6]
        else:
            for base, src in ((0, lo), (6, hi), (12, lo), (18, hi)):
                t24[base:base + 3] = src[3:6]
                t24[base + 3:base + 6] = src[0:3]
        return t24, nm

    in_maps = []
    for core in range(8):
        b, h = core // 2, core % 2
        sl = slice(h * TQ, (h + 1) * TQ)
        q24h, nmq = geo(pq[b][:, sl], True)
        k24h, nmk = geo(pk[b], False)
        nkk = (-(nmk + EPS) / 10.0)[None, :]
        nqq = (-nmq / 10.0).reshape(NI, 128).T       # [128, NI]
        m = {
            "xq": qm[b][:, sl].astype(bf),
            "xkv": kv[b].astype(bf),
            "q24": q24h, "k24": k24h,
            "nkk": nkk.astype(f32), "nqq": nqq.astype(f32),
            "wqg": wqg, "wkg": wkg, "wvt": wvt, "wot": wot,
            "su": su, "bo128": bo128,
            f"nonce{BUILD_ID}": np.zeros((1, 1), f32),
        }
        in_maps.append({k: np.ascontiguousarray(v) for k, v in m.items()})
    return in_maps


def kernel(**inputs):
    if "nc" not in _CACHE:
        _CACHE["nc"] = build_nc()
    nc = _CACHE["nc"]
    in_maps = _host_inputs(inputs)
    res = run_bass_kernel_spmd(nc, in_maps, core_ids=list(range(8)))
    out = np.zeros((B, C, N), np.float32)
    for core in range(8):
        b, h = core // 2, core % 2
        out[b][:, h * TQ:(h + 1) * TQ] = res.results[core]["y"].T
    return out.reshape(B, C, H, W)


# revision 26
# speedup vs baseline: 1.0184x; 1.0167x over previous
"""EpipolarCrossViewAttention TRN2 kernel v2 (8 NeuronCores, data-parallel).

Sharding: core c -> batch b=c//2, query-row half h=c%2 (1152 query rows).
Host does layout + weight folding + ray normalization (O(N), free);
device does all O(N^2) / O(N*C*D) work.

v2 vs baseline:
- bf16 datapath for q/k/v/P/out projections (validated 5.9e-3 rel).
- fp32-exact top-32 selection (hi/lo f32r bias numerator, fp32 gb,
  128-wide chunk max8 + 4-round merge).
- three overlapped phases: T (bias+topk, DVE/Pool-heavy), P
  (projections, PE-heavy), A (attention, PE/Act), interleaved emission
  so engines pipeline across phases; double-buffered pools.
- masked bias mgb = gb + BIG*min(gb-t,0) precomputed in T (bf16),
  applied in A as Act prefill with per-row -max(gb) shift bias.
- row softmax normalization folded into the O_sb copy (Act scale=R).
"""
import os
import numpy as np
import ml_dtypes
import concourse.bass as bass
import concourse.mybir as mybir
import concourse.tile as tile
from concourse import bacc
from concourse.bass_utils import run_bass_kernel_spmd
from concourse.masks import make_identity

F32 = mybir.dt.float32
F32R = mybir.dt.float32r
BF16 = mybir.dt.bfloat16
A = mybir.AluOpType
AF = mybir.ActivationFunctionType

B, C, H, W = 4, 1024, 48, 48
N = H * W            # 2304 keys
TQ = N // 2          # 1152 query rows per core
D = 256
NC_ = C // 128       # 8 c-tiles
NI = TQ // 128       # 9 query row-blocks
NJ = N // 128        # 18 key 128-chunks
EPS = 1e-6
LN_EPS = 1e-5
SCALE = D ** -0.5
BIG = 1.5e9
TOPCW = 128          # topk chunk width -> 18 chunks, top-8 each

_CACHE = {}
BUILD_ID = 206

KCH = [(0, 512), (512, 512), (1024, 512), (1536, 512), (2048, 256)]  # N chunks
CCH = [(0, 512), (512, 512)]                                         # C chunks


def build_nc():
    nc = bacc.Bacc("TRN2", target_bir_lowering=False, debug=False)

    xq_d = nc.dram_tensor("xq", [C, TQ], BF16, kind="ExternalInput")
    xkv_d = nc.dram_tensor("xkv", [C, N], BF16, kind="ExternalInput")
    q24_d = nc.dram_tensor("q24", [24, TQ], F32R, kind="ExternalInput")  # host hi/lo split
    k24_d = nc.dram_tensor("k24", [24, N], F32R, kind="ExternalInput")
    nkk_d = nc.dram_tensor("nkk", [1, N], F32, kind="ExternalInput")    # -(||mk||+eps)/10
    nqq_d = nc.dram_tensor("nqq", [128, NI], F32, kind="ExternalInput")  # -||mq||/10 per row-block
    wqg_d = nc.dram_tensor("wqg", [C, D], BF16, kind="ExternalInput")   # (Wq*g_q).T * scale
    wkg_d = nc.dram_tensor("wkg", [C, D], BF16, kind="ExternalInput")   # (Wk*g_k).T
    wv_d = nc.dram_tensor("wvt", [C, D], BF16, kind="ExternalInput")    # Wv.T
    wo_d = nc.dram_tensor("wot", [D, C], BF16, kind="ExternalInput")    # Wo.T
    su_d = nc.dram_tensor("su", [128, 8], F32, kind="ExternalInput")    # s_q(2) s_k(2) u_q(2) u_k(2)
    bo_d = nc.dram_tensor("bo128", [128, C], BF16, kind="ExternalInput")  # bo + Wo@bv replicated
    y_d = nc.dram_tensor("y", [TQ, C], F32, kind="ExternalOutput")
    nonce_d = nc.dram_tensor(f"nonce{BUILD_ID}", [1, 1], F32, kind="ExternalInput")
    dnonce_d = nc.dram_tensor(f"dnonce{BUILD_ID}", [1, 1], F32, kind="ExternalOutput")
    DBG = bool(os.environ.get("KDBG"))
    if DBG:
        dbg_gb = nc.dram_tensor("dbg_gb", [128, N], F32, kind="ExternalOutput")
        dbg_t = nc.dram_tensor("dbg_t", [128, 8], F32, kind="ExternalOutput")
        dbg_P = nc.dram_tensor("dbg_P", [128, N], BF16, kind="ExternalOutput")
        dbg_S = nc.dram_tensor("dbg_S", [128, 1], F32, kind="ExternalOutput")
        dbg_mgb = nc.dram_tensor("dbg_mgb", [128, N], BF16, kind="ExternalOutput")
        dbg_s1m = nc.dram_tensor("dbg_s1m", [128, N], BF16, kind="ExternalOutput")

    with tile.TileContext(nc) as tc:
      with tc.tile_pool(name="pers", bufs=1) as pers:
        nt = pers.tile([1, 1], F32, tag="nonce_t")
        nc.sync.dma_start(nt[:], nonce_d[:])
        nc.sync.dma_start(dnonce_d[:], nt[:])

        # geometry + bias inputs first in the DMA queue: phase T needs them
        nqq = pers.tile([128, NI], F32, tag="nqq")
        nc.sync.dma_start(nqq[:], nqq_d[:])
        su = pers.tile([128, 8], F32, tag="su")
        nc.sync.dma_start(su[:], su_d[:])
        nkneg_b = pers.tile([128, N], F32, tag="nkneg_b")

        ident_f = pers.tile([128, 128], F32, tag="ident_f")
        make_identity(nc, ident_f[:])
        ident_r = pers.tile([128, 128], F32R, tag="ident_r")
        nc.vector.tensor_copy(ident_r[:], ident_f[:])
        ident_bf = pers.tile([128, 128], BF16, tag="ident_bf")
        nc.vector.tensor_copy(ident_bf[:], ident_f[:])
        invC = pers.tile([128, 1], BF16, tag="invC")
        nc.vector.memset(invC[:], 1.0 / C)
        lneps = pers.tile([1, 1], F32, tag="lneps")
        nc.vector.memset(lneps[:], LN_EPS)

        q_T = [pers.tile([128, TQ], F32R, tag=f"qT{d}", name=f"qT{d}") for d in range(2)]
        k_T = [pers.tile([128, N], F32R, tag=f"kT{d}", name=f"kT{d}") for d in range(2)]
        V = [pers.tile([128, D + 1], BF16, tag=f"V{t}", name=f"V{t}") for t in range(NJ)]
        for t in range(NJ):
            nc.vector.memset(V[t][:, D:D + 1], 1.0)
        mgb = [pers.tile([128, N], BF16, tag=f"mgb{g}", name=f"mgb{g}") for g in range(NI)]
        gmneg = pers.tile([128, NI], F32, tag="gmneg")
        q24 = pers.tile([24, TQ], F32R, tag="q24")
        k24 = pers.tile([24, N], F32R, tag="k24")

        # ---- geometry fully host-side: direct q24/k24 loads ----
        nc.sync.dma_start(q24[:], q24_d[:])
        nc.sync.dma_start(k24[:], k24_d[:])
        with tc.tile_pool(name="geo", bufs=1) as geo:
            nkrow = geo.tile([1, N], F32, tag="nkrow")
            nc.sync.dma_start(nkrow[:], nkk_d[:])
            nc.gpsimd.partition_broadcast(nkneg_b[:], nkrow[0:1, :], channels=128)

        # weights after geometry in the DMA queue
        wqg = [pers.tile([128, D], BF16, tag=f"wqg{c}", name=f"wqg{c}") for c in range(NC_)]
        wkg = [pers.tile([128, D], BF16, tag=f"wkg{c}", name=f"wkg{c}") for c in range(NC_)]
        wv = [pers.tile([128, D], BF16, tag=f"wv{c}", name=f"wv{c}") for c in range(NC_)]
        wo = [pers.tile([128, C], BF16, tag=f"wo{d}", name=f"wo{d}") for d in range(2)]
        for c in range(NC_):
            nc.sync.dma_start(wqg[c][:], wqg_d[c * 128:(c + 1) * 128, :])
            nc.sync.dma_start(wkg[c][:], wkg_d[c * 128:(c + 1) * 128, :])
            nc.sync.dma_start(wv[c][:], wv_d[c * 128:(c + 1) * 128, :])
        for d in range(2):
            nc.sync.dma_start(wo[d][:], wo_d[d * 128:(d + 1) * 128, :])
        bo_bc = pers.tile([128, C], BF16, tag="bo_bc")
        nc.sync.dma_start(bo_bc[:], bo_d[:])

        # ---- phases T (bias+topk) and P (projections), interleaved ----
        NCAND = (N // TOPCW) * 8   # 144

        with tc.tile_pool(name="psT", bufs=3, space="PSUM") as psT, \
             tc.tile_pool(name="tT", bufs=2) as tT, \
             tc.tile_pool(name="tT1", bufs=1) as tT1, \
             tc.tile_pool(name="tTs", bufs=2) as tTs, \
             tc.tile_pool(name="psS", bufs=1, space="PSUM") as psS, \
             tc.tile_pool(name="psA", bufs=2, space="PSUM") as psA, \
             tc.tile_pool(name="psV", bufs=1, space="PSUM") as psV, \
             tc.tile_pool(name="tP", bufs=2) as tP, \
             tc.tile_pool(name="tPs", bufs=2) as tPs:

            def phase_T_front(g):
                a10 = tT.tile([128, N], F32, tag="a10")
                for j0, wd in KCH:
                    pn = psT.tile([128, 512], F32, tag="pn")
                    nc.tensor.matmul(pn[:, :wd], q24[:, g * 128:(g + 1) * 128],
                                     k24[:, j0:j0 + wd], start=True, stop=True)
                    nc.scalar.activation(a10[:, j0:j0 + wd], pn[:, :wd], AF.Abs)
                dneg = tT.tile([128, N], F32, tag="dneg")
                nc.scalar.activation(dneg[:], nkneg_b[:], AF.Identity,
                                     bias=nqq[:, g:g + 1])
                nc.vector.reciprocal(dneg[:], dneg[:])              # rd in place
                nc.gpsimd.tensor_mul(a10[:], a10[:], dneg[:])       # gb in place
                return a10

            def phase_T_topk(g, gb):
                cand = tT1.tile([128, NCAND], F32, tag="cand")
                for i in range(N // TOPCW):
                    nc.vector.max(out=cand[:, i * 8:(i + 1) * 8],
                                  in_=gb[:, i * TOPCW:(i + 1) * TOPCW])
                scr = tT1.tile([128, NCAND], F32, tag="scr")
                m8s = [tTs.tile([128, 8], F32, tag=f"m8{r}", name=f"m8{r}")
                       for r in range(4)]
                cur = cand
                for r in range(4):
                    nc.vector.max(out=m8s[r][:], in_=cur[:])
                    if r < 3:
                        nxt = scr if cur is cand else cand
                        nc.vector.match_replace(out=nxt[:], in_to_replace=m8s[r][:],
                                                in_values=cur[:], imm_value=-3.0e38)
                        cur = nxt
                nc.vector.tensor_scalar(gmneg[:, g:g + 1], m8s[0][:, 0:1],
                                        -1.0, None, op0=A.mult)
                s1m = tT.tile([128, N], BF16, tag="s1m")
                nc.gpsimd.tensor_scalar(s1m[:], gb[:], m8s[3][:, 7:8], 0.0,
                                        op0=A.subtract, op1=A.min)
                if DBG and g == 0:
                    nc.sync.dma_start(dbg_gb[:], gb[:])
                    nc.sync.dma_start(dbg_t[:], m8s[3][:])
                return s1m

            def emit_mgb(g, gb, s1m):
                nc.vector.scalar_tensor_tensor(mgb[g][:], s1m[:], BIG, gb[:],
                                               op0=A.mult, op1=A.add)

            def phase_P(x_d, j0, wd, out_T, s0, u0, with_v, tok0):
                xt = [tP.tile([128, 512], BF16, tag=f"xt{c}", name=f"xt{c}")
                      for c in range(NC_)]
                for c in range(NC_):
                    nc.sync.dma_start(xt[c][:, :wd],
                                      x_d[c * 128:(c + 1) * 128, j0:j0 + wd])
                p_mu = psS.tile([1, 512], F32, tag="p_mu")
                p_m2 = psS.tile([1, 512], F32, tag="p_m2")
                for c in range(NC_):
                    nc.tensor.matmul(p_mu[:, :wd], invC[:], xt[c][:, :wd],
                                     start=(c == 0), stop=(c == NC_ - 1),
                                     skip_group_check=True)
                    xsq = tPs.tile([128, 512], BF16, tag="xsq")
                    nc.vector.tensor_mul(xsq[:, :wd], xt[c][:, :wd], xt[c][:, :wd])
                    nc.tensor.matmul(p_m2[:, :wd], invC[:], xsq[:, :wd],
                                     start=(c == 0), stop=(c == NC_ - 1),
                                     skip_group_check=True)
                st = tPs.tile([1, 512], F32, tag="st")
                nc.scalar.activation(st[:, :wd], p_mu[:, :wd], AF.Square)   # mu^2
                nc.vector.tensor_sub(st[:, :wd], p_m2[:, :wd], st[:, :wd])  # var
                nc.scalar.activation(st[:, :wd], st[:, :wd], AF.Sqrt,
                                     bias=lneps[0:1, 0:1])                  # sd
                rrow = tPs.tile([1, 512], BF16, tag="rrow")
                mrow = tPs.tile([1, 512], BF16, tag="mrow")
                with nc.allow_low_precision(reason="LN scale rows feed bf16 matmul path"):
                    nc.vector.reciprocal(rrow[:, :wd], st[:, :wd])          # rr (bf16)
                    nc.vector.tensor_mul(mrow[:, :wd], rrow[:, :wd], p_mu[:, :wd])
                rr_b = tPs.tile([128, 512], BF16, tag="rr_b")
                nc.gpsimd.partition_broadcast(rr_b[:, :wd], rrow[0:1, :wd], channels=128)
                m_b = tPs.tile([128, 512], BF16, tag="m_b")
                nc.gpsimd.partition_broadcast(m_b[:, :wd], mrow[0:1, :wd], channels=128)
                for dh in range(2):
                    pA = psA.tile([128, 512], F32, tag="pA")
                    for c in range(NC_):
                        nc.tensor.matmul(pA[:, :wd], wqg[c][:, dh * 128:(dh + 1) * 128]
                                         if out_T is q_T else
                                         wkg[c][:, dh * 128:(dh + 1) * 128],
                                         xt[c][:, :wd], start=(c == 0),
                                         stop=(c == NC_ - 1), skip_group_check=True)
                    pAb = tPs.tile([128, 512], BF16, tag="pAb")
                    nc.scalar.activation(pAb[:, :wd], pA[:, :wd], AF.Identity)
                    k1 = tPs.tile([128, 512], BF16, tag="k1")
                    nc.vector.tensor_mul(k1[:, :wd], pAb[:, :wd], rr_b[:, :wd])
                    k2 = tPs.tile([128, 512], BF16, tag="k2")
                    nc.vector.scalar_tensor_tensor(k2[:, :wd], m_b[:, :wd],
                                                   su[:, s0 + dh:s0 + dh + 1],
                                                   k1[:, :wd], op0=A.mult, op1=A.add)
                    nc.scalar.activation(out_T[dh][:, tok0 + j0:tok0 + j0 + wd],
                                         k2[:, :wd], AF.Identity,
                                         bias=su[:, u0 + dh:u0 + dh + 1])
                if with_v:
                    for s in range(wd // 128):
                        t_idx = (j0 + s * 128) // 128
                        pV = psV.tile([128, D], F32, tag="pV")
                        for c in range(NC_):
                            nc.tensor.matmul(pV[:], xt[c][:, s * 128:(s + 1) * 128],
                                             wv[c][:], start=(c == 0),
                                             stop=(c == NC_ - 1),
                                             skip_group_check=True)
                        nc.scalar.activation(V[t_idx][:, :D], pV[:], AF.Identity)

            # interleave: T(g) then one P chunk-unit
            punits = [("kv", j0, wd) for j0, wd in KCH] + \
                     [("q", j0, wd) for j0, wd in [(0, 512), (512, 512), (1024, 128)]]
            pend = None   # (g, gb, s1m) pipelined by one g: mgb emitted late
            for g in range(NI):
                gb = phase_T_front(g)
                if g < len(punits):
                    kind, j0, wd = punits[g]
                    if kind == "kv":
                        phase_P(xkv_d, j0, wd, k_T, 2, 6, True, 0)
                    else:
                        phase_P(xq_d, j0, wd, q_T, 0, 4, False, 0)
                s1m = phase_T_topk(g, gb)
                if pend is not None:
                    emit_mgb(*pend)
                pend = (g, gb, s1m)
            emit_mgb(*pend)

        # ---- phase A: attention ----
        with tc.tile_pool(name="psL", bufs=2, space="PSUM") as psL, \
             tc.tile_pool(name="psTP", bufs=2, space="PSUM") as psTP, \
             tc.tile_pool(name="psT2", bufs=1, space="PSUM") as psT2, \
             tc.tile_pool(name="psO", bufs=1, space="PSUM") as psO, \
             tc.tile_pool(name="psF", bufs=2, space="PSUM") as psF, \
             tc.tile_pool(name="tA", bufs=2) as tA, \
             tc.tile_pool(name="tAs", bufs=2) as tAs:
            for g in range(NI):
                # emgb = exp(mgb - gmax) in place (phase A: exp table resident)
                nc.scalar.activation(mgb[g][:], mgb[g][:], AF.Exp,
                                     bias=gmneg[:, g:g + 1])
                P = tA.tile([128, N], BF16, tag="P")
                for ci, (j0, wd) in enumerate(KCH):
                    pL = psL.tile([128, 512], F32, tag="pL")
                    for dh in range(2):
                        nc.tensor.matmul(pL[:, :wd],
                                         q_T[dh][:, g * 128:(g + 1) * 128],
                                         k_T[dh][:, j0:j0 + wd],
                                         start=(dh == 0), stop=(dh == 1),
                                         skip_group_check=True)
                    nc.scalar.activation(P[:, j0:j0 + wd], pL[:, :wd], AF.Exp)
                    nc.vector.tensor_mul(P[:, j0:j0 + wd], P[:, j0:j0 + wd],
                                         mgb[g][:, j0:j0 + wd])  # mask, bf16 4x
                if DBG and g == 0:
                    nc.sync.dma_start(dbg_P[:], P[:])

                pO = psO.tile([128, D + 1], F32, tag="pO")
                for grp in range(5):  # groups of 4 transposes (last group 2)
                    njg = 4 if grp < 4 else 2
                    ptp = psTP.tile([128, 512], BF16, tag="ptp")
                    for jj in range(njg):
                        j = grp * 4 + jj
                        nc.tensor.transpose(ptp[:, jj * 128:(jj + 1) * 128],
                                            P[:, j * 128:(j + 1) * 128], ident_bf[:])
                    Pt = tAs.tile([128, 512], BF16, tag="Pt")
                    if grp % 2 == 0:
                        nc.scalar.activation(Pt[:, :njg * 128], ptp[:, :njg * 128],
                                             AF.Identity)
                    else:
                        nc.vector.tensor_scalar(Pt[:, :njg * 128], ptp[:, :njg * 128],
                                                1.0, None, op0=A.mult)
                    for jj in range(njg):
                        j = grp * 4 + jj
                        nc.tensor.matmul(pO[:], Pt[:, jj * 128:(jj + 1) * 128],
                                         V[j][:], start=(j == 0), stop=(j == NJ - 1),
                                         skip_group_check=True)
                R = tAs.tile([128, 1], F32, tag="R")
                nc.vector.reciprocal(R[:], pO[:, D:D + 1])
                O_sb = tAs.tile([128, D], BF16, tag="O_sb")
                nc.scalar.activation(O_sb[:], pO[:, :D], AF.Identity, scale=R[:, 0:1])

                ptp2 = psT2.tile([128, 256], BF16, tag="ptp2")
                for dh in range(2):
                    nc.tensor.transpose(ptp2[:, dh * 128:(dh + 1) * 128],
                                        O_sb[:, dh * 128:(dh + 1) * 128], ident_bf[:])
                OT = tAs.tile([128, D], BF16, tag="OT")
                nc.vector.tensor_scalar(OT[:], ptp2[:, 0:D], 1.0, None, op0=A.mult)
                for j0, wd in CCH:
                    pF = psF.tile([128, 512], F32, tag="pF")
                    for dh in range(2):
                        nc.tensor.matmul(pF[:, :wd], OT[:, dh * 128:(dh + 1) * 128],
                                         wo[dh][:, j0:j0 + wd], start=(dh == 0),
                                         stop=(dh == 1), skip_group_check=True)
                    fo = tA.tile([128, 512], F32, tag="fo")
                    nc.vector.tensor_add(fo[:, :wd], pF[:, :wd], bo_bc[:, j0:j0 + wd])
                    nc.sync.dma_start(y_d[g * 128:(g + 1) * 128, j0:j0 + wd], fo[:, :wd])

    nc.finalize()
    return nc


def _host_inputs(inputs):
    f32 = np.float32
    qm = np.asarray(inputs["query_map"], f32).reshape(B, C, N)
    kv = np.asarray(inputs["key_value_map"], f32).reshape(B, C, N)
    pq = np.asarray(inputs["plucker_query"], f32).reshape(B, 6, N)
    pk = np.asarray(inputs["plucker_key"], f32).reshape(B, 6, N)
    Wq, Wk, Wv, Wo = (np.asarray(inputs[k], f32) for k in ("Wq", "Wk", "Wv", "Wo"))
    gq, bq_ln = np.asarray(inputs["ln_q_g"], f32), np.asarray(inputs["ln_q_b"], f32)
    gk, bk_ln = np.asarray(inputs["ln_k_g"], f32), np.asarray(inputs["ln_k_b"], f32)
    bq, bk, bv, bo = (np.asarray(inputs[k], f32) for k in ("bq", "bk", "bv", "bo"))

    bf = ml_dtypes.bfloat16
    wqg = ((Wq * gq[None, :]).T * SCALE).astype(bf)          # [C, D]
    wkg = (Wk * gk[None, :]).T.astype(bf)
    wvt = Wv.T.astype(bf)
    wot = Wo.T.astype(bf)
    u_q = ((Wq @ bq_ln + bq) * SCALE).astype(f32)            # [D]
    u_k = (Wk @ bk_ln + bk).astype(f32)
    s_q = -wqg.astype(f32).sum(axis=0)                       # [D]
    s_k = -wkg.astype(f32).sum(axis=0)
    su = np.zeros((128, 8), f32)
    for dh in range(2):
        su[:, 0 + dh] = s_q[dh * 128:(dh + 1) * 128]
        su[:, 2 + dh] = s_k[dh * 128:(dh + 1) * 128]
        su[:, 4 + dh] = u_q[dh * 128:(dh + 1) * 128]
        su[:, 6 + dh] = u_k[dh * 128:(dh + 1) * 128]
    bo_row = (bo + Wo @ bv).astype(f32)
    bo128 = np.broadcast_to(bo_row[None, :].astype(bf), (128, C))

    # geometry: normalized dirs + moments + norms + f32r hi/lo split
    def rn11(x):
        b = np.ascontiguousarray(x, f32).view(np.uint32)
        return ((b + np.uint32(1 << 11)) & np.uint32(0xFFFFF000)).view(f32)

    def geo(p, qside):  # p [6, M]
        d = p[0:3]; m = p[3:6]
        nd = np.linalg.norm(d, axis=0)
        dn = d / np.maximum(nd, EPS)[None, :]
        nm = np.linalg.norm(m, axis=0)
        p6 = np.concatenate([dn, m], axis=0).astype(f32)
        hi = rn11(p6); lo = (p6 - hi).astype(f32)
        t24 = np.zeros((24, p6.shape[1]), f32)
        if qside:
            for base, src in ((0, lo), (6, lo), (12, hi), (18, hi)):
                t24[base:base + 3] = src[0:3]
                t24[base + 3:base + 6] = src[3:6]
        else:
            for base, src in ((0, lo), (6, hi), (12, lo), (18, hi)):
                t24[base:base + 3] = src[3:6]
                t24[base + 3:base + 6] = src[0:3]
        return t24, nm

    in_maps = []
    for core in range(8):
        b, h = core // 2, core % 2
        sl = slice(h * TQ, (h + 1) * TQ)
        q24h, nmq = geo(pq[b][:, sl], True)
        k24h, nmk = geo(pk[b], False)
        nkk = (-(nmk + EPS) / 10.0)[None, :]
        nqq = (-nmq / 10.0).reshape(NI, 128).T       # [128, NI]
        m = {
            "xq": qm[b][:, sl].astype(bf),
            "xkv": kv[b].astype(bf),
            "q24": q24h, "k24": k24h,
            "nkk": nkk.astype(f32), "nqq": nqq.astype(f32),
            "wqg": wqg, "wkg": wkg, "wvt": wvt, "wot": wot,
            "su": su, "bo128": bo128,
            f"nonce{BUILD_ID}": np.zeros((1, 1), f32),
        }
        in_maps.append({k: np.ascontiguousarray(v) for k, v in m.items()})
    return in_maps


def kernel(**inputs):
    if "nc" not in _CACHE:
        _CACHE["nc"] = build_nc()
    nc = _CACHE["nc"]
    in_maps = _host_inputs(inputs)
    res = run_bass_kernel_spmd(nc, in_maps, core_ids=list(range(8)))
    out = np.zeros((B, C, N), np.float32)
    for core in range(8):
        b, h = core // 2, core % 2
        out[b][:, h * TQ:(h + 1) * TQ] = res.results[core]["y"].T
    return out.reshape(B, C, H, W)
